# revision 3
# baseline (speedup 1.0000x reference)
"""Multi-head attention (B=8, S=1024, D=1024, H=16) on 8 Trainium2 NeuronCores.

Sharding: data-parallel over batch — core b computes batch element b end to
end (no collectives).  Weights are replicated to every core.

Per-core dataflow (all layouts chosen so the PE contracts over partitions and
no partition-dim broadcast/reduction is ever needed):

  1. x^T   [D,S]   via 64 PE transposes of x tiles (f32, identity matmul)
  2. Q^T,K^T [D,S] = WQ/WK as lhsT, x^T as rhs (f32r matmuls, full PE rate);
                     per-partition bias bQ/bK applied by the ACT copy
  3. V     [S,D]   = x^T tiles as lhsT, WV as rhs (f32r); bias bV folded in
                     as a K=1 matmul (ones-row ⊗ bV); stored bf16 with a
                     ones column per head: V̂_h = [V_h | 1]  (65 cols/head)
  4. per head h:  st[k,q] = K_h^T.T @ Q_h^T  (f32r; head pairs share a
                     partition tile and run on disjoint PE row groups)
                  E = exp(0.125*st - 30*mask_k)   (ACT, bf16, bias = per-
                     partition mask vector; masked keys get weight ~1e-13)
                  AV: out^T[0:64] = unnormalized head output,
                      row 64      = r[q] = sum_k E  (from the ones column)
                  normalize: rc = 1/r (DVE reciprocal, exact), zero masked-q
                      columns (x (1-mask_q) at partition 0), broadcast across
                      partitions via a K=1 ones matmul, multiply.
                  Y^T columns for masked queries are zero; they are replaced
                      at the end (see 6).  Odd heads land at partitions 64:128
                      of the Y^T pair tile via an SBUF->SBUF DMA.
  5. u = mean_k V̂ (tiny N=1 matmuls) -> w0 = u @ WO (f32r)
  6. O = Y^T tiles as lhsT vs WO (f32r) + ones⊗bO + mask_q⊗w0 (K=1 matmuls).
     For a masked query q the reference softmax is uniform (all scores equal
     -1e9), so out[q] = mean_k(V) @ WO + bO = w0 + bO — exactly what the
     zeroed Y rows + bias matmuls produce.

The walrus build in this container accepts only ONE semaphore wait per
instruction, so after tracing we split multi-wait instructions into
single-wait same-engine NoOp prefixes (order on an engine queue enforces the
same semantics).
"""

import numpy as np

import concourse.bass as bass
import concourse.mybir as mybir
from concourse.tile import TileContext
from concourse.bass_utils import run_bass_kernel_spmd

AF = mybir.ActivationFunctionType
F32 = mybir.dt.float32
F32R = mybir.dt.float32r
BF16 = mybir.dt.bfloat16

B, S, D, H = 8, 1024, 1024, 16
DH = D // H          # 64
P = 128
ST = S // P          # 8 s-tiles
DT = D // P          # 8 d-tiles
NEG = -30.0          # replaces -1e9; exp(-30)~1e-13 vs exact 0, and keeps
                     # fully-masked rows uniform after the masked-q fixup
N_CORES = 8

_nop_counter = [0]


def _split_multi_waits(nc):
    for bb in nc.main_func.blocks:
        raw = bb.bb if hasattr(bb, "bb") else bb
        changed = False
        new_list = []
        for ins in raw.instructions:
            si = ins.sync_info
            waits = list(si.on_wait) if si is not None else []
            if len(waits) > 1:
                changed = True
                for w in waits[:-1]:
                    _nop_counter[0] += 1
                    nop = mybir.InstNoOp(
                        name=f"legal_wait_nop_{_nop_counter[0]}", engine=ins.engine
                    )
                    nop.sync_info = mybir.SyncInfo(on_wait=[w], on_update=[])
                    new_list.append(nop)
                ins.sync_info = mybir.SyncInfo(
                    on_wait=[waits[-1]], on_update=list(si.on_update)
                )
            new_list.append(ins)
        if changed:
            raw.instructions = new_list


def _make_identity(nc, ident_f32):
    nc.gpsimd.memset(ident_f32[:], 0.0)
    nc.gpsimd.affine_select(
        out=ident_f32[:],
        in_=ident_f32[:],
        compare_op=mybir.AluOpType.not_equal,
        fill=1.0,
        base=0,
        pattern=[[-1, P]],
        channel_multiplier=1,
    )


def _build():
    nc = bass.Bass("TRN2", num_devices=N_CORES)

    x_d = nc.dram_tensor("x", [S, D], F32, kind="ExternalInput")
    wq_d = nc.dram_tensor("WQ", [D, D], F32R, kind="ExternalInput")
    wk_d = nc.dram_tensor("WK", [D, D], F32R, kind="ExternalInput")
    wv_d = nc.dram_tensor("WV", [D, D], F32R, kind="ExternalInput")
    wo_d = nc.dram_tensor("WO", [D, D], F32R, kind="ExternalInput")
    bq_d = nc.dram_tensor("bQ", [D], F32, kind="ExternalInput")
    bk_d = nc.dram_tensor("bK", [D], F32, kind="ExternalInput")
    bv_d = nc.dram_tensor("bV", [D], F32R, kind="ExternalInput")
    bo_d = nc.dram_tensor("bO", [D], F32R, kind="ExternalInput")
    # host-prepared mask vectors
    mv_d = nc.dram_tensor("mvec", [S], F32, kind="ExternalInput")     # -30*mask
    om_d = nc.dram_tensor("onem", [S], F32, kind="ExternalInput")     # 1-mask
    mf_d = nc.dram_tensor("maskf", [S], F32R, kind="ExternalInput")   # mask
    out_d = nc.dram_tensor("out", [S, D], F32, kind="ExternalOutput")

    with TileContext(nc) as tc:
        with tc.tile_pool(name="misc", bufs=1) as misc, \
             tc.tile_pool(name="w", bufs=1) as wpool, \
             tc.tile_pool(name="yt", bufs=1) as ytp:

            ident = misc.tile([P, P], F32, tag="ident")
            _make_identity(nc, ident)
            onesf = misc.tile([1, P], F32, tag="onesf")
            nc.vector.memset(onesf[:], 1.0)
            onesr = misc.tile([1, P], F32R, tag="onesr")
            nc.vector.tensor_copy(onesr[:], onesf[:])
            onek = misc.tile([P, 1], BF16, tag="onek")
            nc.vector.memset(onek[:], 1.0 / 1024.0)

            bq_sb = misc.tile([P, DT], F32, tag="bq")
            nc.sync.dma_start(bq_sb[:], bq_d.rearrange("(c p) -> p c", p=P))
            bk_sb = misc.tile([P, DT], F32, tag="bk")
            nc.sync.dma_start(bk_sb[:], bk_d.rearrange("(c p) -> p c", p=P))
            bv_row = misc.tile([1, D], F32R, tag="bv")
            nc.sync.dma_start(bv_row[:], bv_d[None, :])
            bo_row = misc.tile([1, D], F32R, tag="bo")
            nc.sync.dma_start(bo_row[:], bo_d[None, :])
            mvec = misc.tile([P, ST], F32, tag="mvec")
            nc.sync.dma_start(mvec[:], mv_d.rearrange("(t p) -> p t", p=P))
            onem_row = misc.tile([1, S], F32, tag="onem")
            nc.sync.dma_start(onem_row[:], om_d[None, :])
            mf_row = misc.tile([1, S], F32R, tag="maskf")
            nc.sync.dma_start(mf_row[:], mf_d[None, :])

            ucol = misc.tile([P, DT], F32R, tag="ucol")
            w0_row = misc.tile([1, D], F32R, tag="w0")
            yt = ytp.tile([P, DT, S], F32R, tag="yt")

            with tc.tile_pool(name="qk", bufs=1) as qkp, \
                 tc.tile_pool(name="vh", bufs=1) as vhp:
                qt = qkp.tile([P, DT, S], F32R, tag="qt")
                kt = qkp.tile([P, DT, S], F32R, tag="kt")
                vhat = vhp.tile([P, ST, H, DH + 1], BF16, tag="vhat")
                nc.vector.memset(vhat[:, :, :, DH], 1.0)

                # ---- phases 1-4: x^T, Q^T, K^T, V ----
                with tc.tile_pool(name="xt", bufs=1) as xtp, \
                     tc.tile_pool(name="pst", bufs=2, space="PSUM") as pst:
                    xT = xtp.tile([P, DT, S], F32R, tag="xT")
                    with tc.tile_pool(name="xin", bufs=2) as xin:
                        for i in range(ST):
                            x_t = xin.tile([P, D], F32, tag="x")
                            nc.sync.dma_start(
                                x_t[:], x_d[i * P:(i + 1) * P, :])
                            for j in range(DT):
                                tp = pst.tile([P, P], F32, tag="tr")
                                nc.tensor.transpose(
                                    tp[:], x_t[:, j * P:(j + 1) * P], ident[:])
                                nc.vector.tensor_copy(
                                    xT[:, j, i * P:(i + 1) * P], tp[:])

                    # Q^T / K^T: per d'-chunk c accumulate over d tiles
                    for (w_d, b_sb, dst) in ((wq_d, bq_sb, qt), (wk_d, bk_sb, kt)):
                        w_sb = wpool.tile([P, DT, D], F32R, tag="w")
                        nc.sync.dma_start(
                            w_sb[:], w_d.rearrange("(t p) n -> p t n", p=P))
                        for c in range(DT):
                            ps = pst.tile([P, S], F32, tag="proj")
                            for t in range(DT):
                                for h2 in range(2):
                                    nc.tensor.matmul(
                                        ps[:, h2 * 512:(h2 + 1) * 512],
                                        w_sb[:, t, c * P:(c + 1) * P],
                                        xT[:, t, h2 * 512:(h2 + 1) * 512],
                                        start=(t == 0), stop=(t == DT - 1))
                            nc.scalar.activation(
                                dst[:, c, :], ps[:], AF.Identity,
                                bias=b_sb[:, c:c + 1], scale=1.0)

                    # V: per s-chunk accumulate over d tiles; += ones⊗bV;
                    # write bf16 head-strided with the ones column
                    w_sb = wpool.tile([P, DT, D], F32R, tag="w")
                    nc.sync.dma_start(
                        w_sb[:], wv_d.rearrange("(t p) n -> p t n", p=P))
                    for c in range(ST):
                        ps = pst.tile([P, S], F32, tag="proj")
                        for t in range(DT):
                            for h2 in range(2):
                                nc.tensor.matmul(
                                    ps[:, h2 * 512:(h2 + 1) * 512],
                                    xT[:, t, c * P:(c + 1) * P],
                                    w_sb[:, t, h2 * 512:(h2 + 1) * 512],
                                    start=(t == 0), stop=False)
                        for h2 in range(2):
                            nc.tensor.matmul(
                                ps[:, h2 * 512:(h2 + 1) * 512],
                                onesr[0:1, 0:P],
                                bv_row[0:1, h2 * 512:(h2 + 1) * 512],
                                start=False, stop=True)
                        nc.vector.tensor_copy(
                            vhat[:, c, :, 0:DH],
                            ps[:].rearrange("p (h e) -> p h e", h=H))

                # ---- phase 5: attention, head pairs ----
                wo_sb = wpool.tile([P, DT, D], F32R, tag="w")
                nc.sync.dma_start(
                    wo_sb[:], wo_d.rearrange("(t p) n -> p t n", p=P))

                with tc.tile_pool(name="epool", bufs=6) as epool, \
                     tc.tile_pool(name="scp", bufs=1) as scp, \
                     tc.tile_pool(name="psst", bufs=2, space="PSUM") as psst, \
                     tc.tile_pool(name="psav", bufs=2, space="PSUM") as psav:
                    for pr in range(H // 2):
                        avs = []
                        for sub in range(2):
                            avs.append(psav.tile([DH + 1, S], F32, tag="av", name=f"av_{pr}_{sub}"))
                        for kk in range(ST):
                            e_ts = []
                            for sub in range(2):
                                h = 2 * pr + sub
                                ro = DH * sub
                                stp = psst.tile([P, S], F32, tag="st")
                                for qc in range(2):
                                    nc.tensor.matmul(
                                        stp[:, qc * 512:(qc + 1) * 512],
                                        kt[ro:ro + DH, pr, kk * P:(kk + 1) * P],
                                        qt[ro:ro + DH, pr, qc * 512:(qc + 1) * 512],
                                        start=True, stop=True)
                                e_t = epool.tile([P, S], BF16, tag="E", name=f"e_{pr}_{kk}_{sub}")
                                nc.scalar.activation(
                                    e_t[:], stp[:], AF.Exp,
                                    bias=mvec[:, kk:kk + 1], scale=0.125)
                                e_ts.append(e_t)
                            for sub in range(2):
                                h = 2 * pr + sub
                                for qc in range(2):
                                    nc.tensor.matmul(
                                        avs[sub][:, qc * 512:(qc + 1) * 512],
                                        vhat[:, kk, h, :],
                                        e_ts[sub][:, qc * 512:(qc + 1) * 512],
                                        start=(kk == 0), stop=(kk == ST - 1))
                        # normalize + masked-q zeroing + partition placement
                        for sub in range(2):
                            av = avs[sub]
                            rc64 = scp.tile([DH + 1, S], F32R, tag="rc64")
                            with nc.allow_low_precision(reason="softmax recip"):
                                nc.vector.reciprocal(
                                    rc64[DH:DH + 1, :], av[DH:DH + 1, :])
                            rc0 = scp.tile([1, S], F32R, tag="rc0")
                            nc.sync.dma_start(rc0[:], rc64[DH:DH + 1, :])
                            # zero masked-q columns: rc0 *= (1-mask_q)
                            nc.vector.tensor_tensor(
                                rc0[:], rc0[:], onem_row[:],
                                mybir.AluOpType.mult)
                            bc = psst.tile([P, S], F32, tag="st")
                            for qc in range(2):
                                nc.tensor.matmul(
                                    bc[0:DH, qc * 512:(qc + 1) * 512],
                                    onesr[0:1, 0:DH],
                                    rc0[0:1, qc * 512:(qc + 1) * 512],
                                    start=True, stop=True)
                            rcb = scp.tile([DH, S], F32, tag="rcb",
                                           name=f"rcb_{pr}_{sub}")
                            nc.vector.tensor_copy(rcb[:], bc[0:DH, :])
                            if sub == 0:
                                nc.vector.tensor_tensor(
                                    yt[0:DH, pr, :], av[0:DH, :], rcb[:],
                                    mybir.AluOpType.mult)
                            else:
                                scr = scp.tile([DH, S], F32R, tag="scr")
                                nc.vector.tensor_tensor(
                                    scr[:], av[0:DH, :], rcb[:],
                                    mybir.AluOpType.mult)
                                nc.sync.dma_start(
                                    yt[DH:P, pr, :], scr[:])

                # ---- phase 5b: u = mean_k V̂, w0 = u @ WO ----
                with tc.tile_pool(name="psu", bufs=2, space="PSUM") as psu:
                    for t in range(DT):
                        up = psu.tile([P, 1], F32, tag="u")
                        for kk in range(ST):
                            nc.tensor.matmul(
                                up[0:DH, :], vhat[:, kk, 2 * t, 0:DH],
                                onek[:], start=(kk == 0), stop=(kk == ST - 1))
                        for kk in range(ST):
                            nc.tensor.matmul(
                                up[DH:P, :], vhat[:, kk, 2 * t + 1, 0:DH],
                                onek[:], start=(kk == 0), stop=(kk == ST - 1),
                                tile_position=(0, DH))
                        nc.vector.tensor_copy(ucol[:, t:t + 1], up[:])
                    wp = psu.tile([1, D], F32, tag="w0ps")
                    for t in range(DT):
                        for h2 in range(2):
                            nc.tensor.matmul(
                                wp[0:1, h2 * 512:(h2 + 1) * 512],
                                ucol[:, t:t + 1],
                                wo_sb[:, t, h2 * 512:(h2 + 1) * 512],
                                start=(t == 0), stop=(t == DT - 1))
                    nc.vector.tensor_copy(w0_row[:], wp[:])

            # ---- phase 6: O = Y @ WO + bO (+ masked-row fixup) ----
            with tc.tile_pool(name="pso", bufs=2, space="PSUM") as pso, \
                 tc.tile_pool(name="outp", bufs=3) as outp:
                for c in range(ST):
                    ps = pso.tile([P, D], F32, tag="o")
                    for t in range(DT):
                        for h2 in range(2):
                            nc.tensor.matmul(
                                ps[:, h2 * 512:(h2 + 1) * 512],
                                yt[:, t, c * P:(c + 1) * P],
                                wo_sb[:, t, h2 * 512:(h2 + 1) * 512],
                                start=(t == 0), stop=False)
                    for h2 in range(2):
                        nc.tensor.matmul(
                            ps[:, h2 * 512:(h2 + 1) * 512],
                            onesr[0:1, 0:P],
                            bo_row[0:1, h2 * 512:(h2 + 1) * 512],
                            start=False, stop=False)
                        nc.tensor.matmul(
                            ps[:, h2 * 512:(h2 + 1) * 512],
                            mf_row[0:1, c * P:(c + 1) * P],
                            w0_row[0:1, h2 * 512:(h2 + 1) * 512],
                            start=False, stop=True)
                    o_sb = outp.tile([P, D], F32, tag="osb")
                    nc.scalar.activation(o_sb[:], ps[:], AF.Copy)
                    nc.sync.dma_start(out_d[c * P:(c + 1) * P, :], o_sb[:])

    _split_multi_waits(nc)
    return nc


_cached = {}

TRACE = False          # set by test.py to capture an NTFF profile
LAST_RESULTS = None    # BassKernelResults of the most recent run


def kernel(**inputs):
    ins = {k: np.asarray(v) for k, v in inputs.items()}
    x = ins["x"].astype(np.float32)            # [B, S, D]
    mask = ins["mask"].astype(bool)            # [B, S]
    if "nc" not in _cached:
        _cached["nc"] = _build()
    nc = _cached["nc"]

    mask_f = mask.astype(np.float32)
    weights = {
        "WQ": np.ascontiguousarray(ins["WQ"].astype(np.float32)),
        "WK": np.ascontiguousarray(ins["WK"].astype(np.float32)),
        "WV": np.ascontiguousarray(ins["WV"].astype(np.float32)),
        "WO": np.ascontiguousarray(ins["WO"].astype(np.float32)),
        "bQ": np.ascontiguousarray(ins["bQ"].astype(np.float32)),
        "bK": np.ascontiguousarray(ins["bK"].astype(np.float32)),
        "bV": np.ascontiguousarray(ins["bV"].astype(np.float32)),
        "bO": np.ascontiguousarray(ins["bO"].astype(np.float32)),
    }
    in_maps = []
    for b in range(N_CORES):
        m = dict(weights)
        m["x"] = np.ascontiguousarray(x[b])
        m["mvec"] = np.ascontiguousarray(NEG * mask_f[b])
        m["onem"] = np.ascontiguousarray(1.0 - mask_f[b])
        m["maskf"] = np.ascontiguousarray(mask_f[b])
        in_maps.append(m)

    res = run_bass_kernel_spmd(
        nc, in_maps, core_ids=list(range(N_CORES)), trace=TRACE)
    globals()["LAST_RESULTS"] = res
    return np.stack([r["out"] for r in res.results], axis=0)

